# revision 8
# baseline (speedup 1.0000x reference)
"""DCRNN Trainium2 kernel.

Model (see problem): B=128, T=24, N=512 nodes, F_IN=2, H=128, K=2 diffusion
steps, 2 stacked DCGRU layers, decoder MLP to HORIZON=12.

Strategy:
  - Data-parallel over batch: 8 cores x 16 local batch (BL).
  - All state stays in SBUF across the 24 recurrent steps (no HBM in loop).
  - Two on-chip layouts per activation tensor:
      nmb ("node-major"):  [128 node-part, mt-block*BL*128 + b*128 + feat]
      fm2 ("feat-major"):  per-b tiles [128 feat-part, 512 node]
    Diffusion contracts over nodes -> needs node-major lhsT;
    gate GEMMs contract over features -> need feat-major rhs.
    Conversions via DMA-xbar transposes of [128,128] blocks (off-engine).
  - L^2 is precomputed on host (L2 = L@L), so each diffusion trio needs only
    two matmul applications (rhs = L^T and (L^2)^T), both fm2-producing.
  - x_diff (input diffusion features) precomputed on host (0.8% of FLOPs),
    fed per-step as a K=6 fm2 block; gate weights packed to match.
  - bf16 matmul operands, fp32 PSUM accumulation, fp32 biases.
"""

import numpy as np
import ml_dtypes
from contextlib import ExitStack

B, T, N, F_IN = 128, 24, 512, 2
H, KDIFF, LAYERS, HORIZON = 128, 2, 2, 12
NCORES = 8
BL = B // NCORES          # 16 local batch per core
NT = N // 128             # 4 node tiles
COLS = BL * N             # 8192 fm2 columns (b*512 + n)
BF16 = ml_dtypes.bfloat16

_BUILD_CACHE = {}


def _build(t_steps=T):
    import concourse.mybir as mybir
    import concourse.tile as tile
    from concourse import bacc

    f32 = mybir.dt.float32
    bf16 = mybir.dt.bfloat16
    AF = mybir.ActivationFunctionType

    nc = bacc.Bacc("TRN2", target_bir_lowering=False, debug=False,
                   enable_asserts=False, num_devices=NCORES)

    # ---- DRAM I/O ----
    xd = nc.dram_tensor("xd", [t_steps, 6, COLS], bf16, kind="ExternalInput").ap()
    lt = nc.dram_tensor("lt", [128, NT * N], bf16, kind="ExternalInput").ap()
    l2t = nc.dram_tensor("l2t", [128, NT * N], bf16, kind="ExternalInput").ap()
    w0x = nc.dram_tensor("w0x", [6, 3 * H], bf16, kind="ExternalInput").ap()
    w0h = nc.dram_tensor("w0h", [128, 9 * H], bf16, kind="ExternalInput").ap()
    w1 = nc.dram_tensor("w1", [128, 18 * H], bf16, kind="ExternalInput").ap()
    b0 = nc.dram_tensor("b0", [128, 3], f32, kind="ExternalInput").ap()
    b1 = nc.dram_tensor("b1", [128, 3], f32, kind="ExternalInput").ap()
    wd1 = nc.dram_tensor("wd1", [128, H], bf16, kind="ExternalInput").ap()
    wd2 = nc.dram_tensor("wd2", [128, HORIZON], bf16, kind="ExternalInput").ap()
    bd1 = nc.dram_tensor("bd1", [128, 1], f32, kind="ExternalInput").ap()
    bd2 = nc.dram_tensor("bd2", [HORIZON, 1], f32, kind="ExternalInput").ap()
    y = nc.dram_tensor("y", [HORIZON, COLS], f32, kind="ExternalOutput").ap()

    with tile.TileContext(nc) as tc, ExitStack() as ctx:
        consts = ctx.enter_context(tc.tile_pool(name="consts", bufs=1))
        state = ctx.enter_context(tc.tile_pool(name="state", bufs=2))
        work = ctx.enter_context(tc.tile_pool(name="work", bufs=3))
        xkp = ctx.enter_context(tc.tile_pool(name="xkp", bufs=2))
        pst = ctx.enter_context(tc.tile_pool(name="pst", bufs=4, space="PSUM"))
        psg = ctx.enter_context(tc.tile_pool(name="psg", bufs=4, space="PSUM"))

        # ---- load constants ----
        lt_sb = consts.tile([128, NT * N], bf16)
        nc.sync.dma_start(lt_sb, lt)
        l2t_sb = consts.tile([128, NT * N], bf16)
        nc.sync.dma_start(l2t_sb, l2t)
        w0x_sb = consts.tile([6, 3 * H], bf16)
        nc.sync.dma_start(w0x_sb, w0x)
        w0h_sb = consts.tile([128, 9 * H], bf16)
        nc.sync.dma_start(w0h_sb, w0h)
        w1_sb = consts.tile([128, 18 * H], bf16)
        nc.sync.dma_start(w1_sb, w1)
        b0_sb = consts.tile([128, 3], f32)
        nc.sync.dma_start(b0_sb, b0)
        b1_sb = consts.tile([128, 3], f32)
        nc.sync.dma_start(b1_sb, b1)
        wd1_sb = consts.tile([128, H], bf16)
        nc.sync.dma_start(wd1_sb, wd1)
        wd2_sb = consts.tile([128, HORIZON], bf16)
        nc.sync.dma_start(wd2_sb, wd2)
        bd1_sb = consts.tile([128, 1], f32)
        nc.sync.dma_start(bd1_sb, bd1)
        bd2_sb = consts.tile([HORIZON, 1], f32)
        nc.sync.dma_start(bd2_sb, bd2)

        # ---- recurrent state (node-major) ----
        # layout: [128 node-part, mt*BL*128 + b*128 + feat]
        h0n = state.tile([128, NT * BL * 128], bf16, tag="h0n")
        h1n = state.tile([128, NT * BL * 128], bf16, tag="h1n")
        nc.vector.memset(h0n, 0.0)
        nc.vector.memset(h1n, 0.0)

        def nmb_blk(hn, mt, b):
            return hn[:, mt * (BL * 128) + b * 128: mt * (BL * 128) + (b + 1) * 128]

        def tp_to_fm2(dst, hn, b):
            # dst [128 feat, 512 node] <- transpose of 4 nmb blocks
            for j in range(NT):
                nc.sync.dma_start_transpose(
                    dst[:, j * 128:(j + 1) * 128], nmb_blk(hn, j, b))

        def tp_to_nmb(hn, src, b):
            for j in range(NT):
                nc.sync.dma_start_transpose(nmb_blk(hn, j, b),
                                            src[:, j * 128:(j + 1) * 128])

        def trio(blocks, tag):
            """Diffuse the nmb source given by its 4 [128,128] m-tile blocks.
            Returns (L z, L^2 z) as fm2 bf16 [128, 512] tiles."""
            outs = []
            for mat, sfx, ev in ((lt_sb, "a", "v"), (l2t_sb, "b", "s")):
                ps = pst.tile([128, N], f32, tag="pst", name=f"ps_{tag}{sfx}")
                for mt in range(NT):
                    nc.tensor.matmul(ps, blocks[mt],
                                     mat[:, mt * N:(mt + 1) * N],
                                     start=(mt == 0), stop=(mt == NT - 1))
                z = work.tile([128, N], bf16, tag=f"{tag}{sfx}",
                              name=f"{tag}{sfx}")
                if ev == "v":
                    nc.vector.tensor_copy(z, ps)
                else:
                    nc.scalar.copy(z, ps)
                outs.append(z)
            return outs

        def gate(feats, bias_ap, func, tag):
            """feats: list of (lhsT_w_ap, rhs_feat_ap). Returns bf16 tile."""
            ps = psg.tile([128, N], f32, tag="psg", name=f"ps_{tag}")
            nfe = len(feats)
            for i, (wap, fap) in enumerate(feats):
                nc.tensor.matmul(ps, wap, fap,
                                 start=(i == 0), stop=(i == nfe - 1))
            g = work.tile([128, N], bf16, tag=tag, name=tag)
            nc.scalar.activation(g, ps, func, bias=bias_ap)
            return g

        AFSig, AFTanh = AF.Sigmoid, AF.Tanh

        for t in range(t_steps):
            xk = xkp.tile([6, COLS], bf16, tag="xk", name="xk")
            nc.sync.dma_start(xk, xd[t])

            h0n_new = state.tile([128, NT * BL * 128], bf16, tag="h0n",
                                 name="h0n_new")
            h1n_new = state.tile([128, NT * BL * 128], bf16, tag="h1n",
                                 name="h1n_new")

            for layer in range(LAYERS):
                hn = h0n if layer == 0 else h1n
                hn_new = h0n_new if layer == 0 else h1n_new
                wsb = w0h_sb if layer == 0 else w1_sb
                bsb = b0_sb if layer == 0 else b1_sb

                for b in range(BL):
                    hf = work.tile([128, N], bf16, tag="hf", name="hf")
                    tp_to_fm2(hf, hn, b)
                    lh, l2h = trio([nmb_blk(hn, mt, b) for mt in range(NT)],
                                   "lh")
                    if layer == 0:
                        # x features come precomputed via xk (K=6 block)
                        xfeat = [(w0x_sb[:, g * H:(g + 1) * H],
                                  xk[:, b * N:(b + 1) * N]) for g in range(3)]
                        hblk = lambda g, j: wsb[:, (g * 3 + j) * H:(g * 3 + j + 1) * H]
                        r = gate([xfeat[0],
                                  (hblk(0, 0), hf), (hblk(0, 1), lh),
                                  (hblk(0, 2), l2h)],
                                 bsb[:, 0:1], AFSig, "r")
                        u = gate([xfeat[1],
                                  (hblk(1, 0), hf), (hblk(1, 1), lh),
                                  (hblk(1, 2), l2h)],
                                 bsb[:, 1:2], AFSig, "u")
                    else:
                        xf = work.tile([128, N], bf16, tag="xf", name="xf")
                        tp_to_fm2(xf, h0n_new, b)
                        lx, l2x = trio(
                            [nmb_blk(h0n_new, mt, b) for mt in range(NT)],
                            "lx")
                        hblk = lambda g, j: wsb[:, (g * 6 + j) * H:(g * 6 + j + 1) * H]
                        r = gate([(hblk(0, 0), xf), (hblk(0, 1), lx),
                                  (hblk(0, 2), l2x), (hblk(0, 3), hf),
                                  (hblk(0, 4), lh), (hblk(0, 5), l2h)],
                                 bsb[:, 0:1], AFSig, "r")
                        u = gate([(hblk(1, 0), xf), (hblk(1, 1), lx),
                                  (hblk(1, 2), l2x), (hblk(1, 3), hf),
                                  (hblk(1, 4), lh), (hblk(1, 5), l2h)],
                                 bsb[:, 1:2], AFSig, "u")

                    rh = work.tile([128, N], bf16, tag="rh", name="rh")
                    nc.vector.tensor_mul(rh, r, hf)
                    rhn = work.tile([128, N], bf16, tag="rhn", name="rhn")
                    for j in range(NT):
                        nc.sync.dma_start_transpose(
                            rhn[:, j * 128:(j + 1) * 128],
                            rh[:, j * 128:(j + 1) * 128])
                    lrh, l2rh = trio(
                        [rhn[:, mt * 128:(mt + 1) * 128] for mt in range(NT)],
                        "lr")
                    if layer == 0:
                        c = gate([xfeat[2],
                                  (hblk(2, 0), rh), (hblk(2, 1), lrh),
                                  (hblk(2, 2), l2rh)],
                                 bsb[:, 2:3], AFTanh, "c")
                    else:
                        c = gate([(hblk(2, 0), xf), (hblk(2, 1), lx),
                                  (hblk(2, 2), l2x), (hblk(2, 3), rh),
                                  (hblk(2, 4), lrh), (hblk(2, 5), l2rh)],
                                 bsb[:, 2:3], AFTanh, "c")

                    # h' = h + u*(c-h)
                    d = work.tile([128, N], bf16, tag="d", name="d")
                    nc.vector.tensor_sub(d, c, hf)
                    e = work.tile([128, N], bf16, tag="e", name="e")
                    nc.vector.tensor_mul(e, u, d)
                    hpf = work.tile([128, N], bf16, tag="hpf", name="hpf")
                    nc.gpsimd.tensor_add(hpf, hf, e)
                    tp_to_nmb(hn_new, hpf, b)

            h0n, h1n = h0n_new, h1n_new

        # ---- decoder ----
        for b in range(BL):
            hf = work.tile([128, N], bf16, tag="hf", name="hf_dec")
            tp_to_fm2(hf, h1n, b)
            ps = psg.tile([128, N], f32, tag="psg", name="ps_dec1")
            nc.tensor.matmul(ps, wd1_sb, hf, start=True, stop=True)
            hid = work.tile([128, N], bf16, tag="hid", name="hid")
            nc.scalar.activation(hid, ps, AF.Relu, bias=bd1_sb[:, 0:1])
            ps2 = psg.tile([HORIZON, N], f32, tag="psg", name="ps_dec2")
            nc.tensor.matmul(ps2, wd2_sb, hid, start=True, stop=True)
            yo = work.tile([HORIZON, N], f32, tag="yo", name="yo")
            nc.scalar.activation(yo, ps2, AF.Identity, bias=bd2_sb[:, 0:1])
            nc.sync.dma_start(y[:, b * N:(b + 1) * N], yo)

    nc.compile()
    return nc


def _host_prep(x, L, params, dec, t_steps=T):
    """Numpy preprocessing -> per-core in_maps."""
    x = np.asarray(x, dtype=np.float32)
    L = np.asarray(L, dtype=np.float32)
    Ld = L.astype(np.float64)
    L2 = (Ld @ Ld).astype(np.float32)

    lt_h = np.concatenate([L.T[mt * 128:(mt + 1) * 128] for mt in range(NT)],
                          axis=1).astype(BF16)           # [128, 4*512]
    l2t_h = np.concatenate([L2.T[mt * 128:(mt + 1) * 128] for mt in range(NT)],
                           axis=1).astype(BF16)

    # x_diff on host: feats [B, T, N, 6] = [x, Lx, L2x]
    xt = x[:, :t_steps]                                  # [B, t, N, 2]
    xp = xt.transpose(2, 0, 1, 3).reshape(N, -1)         # [N, B*t*2]
    lx = (L @ xp).reshape(N, B, t_steps, 2).transpose(1, 2, 0, 3)
    l2x = (L2 @ xp).reshape(N, B, t_steps, 2).transpose(1, 2, 0, 3)
    feats = np.concatenate([xt, lx, l2x], axis=-1)       # [B, t, N, 6]

    W0 = [np.asarray(params[0][k], np.float32) for k in ("Wr", "Wu", "Wc")]
    bias0 = [np.asarray(params[0][k], np.float32) for k in ("br", "bu", "bc")]
    W1 = [np.asarray(params[1][k], np.float32) for k in ("Wr", "Wu", "Wc")]
    bias1 = [np.asarray(params[1][k], np.float32) for k in ("br", "bu", "bc")]

    w0x_h = np.concatenate([W[0:6] for W in W0], axis=1).astype(BF16)  # [6, 384]
    w0h_h = np.concatenate(
        [W0[g][6 + j * 128: 6 + (j + 1) * 128] for g in range(3) for j in range(3)],
        axis=1).astype(BF16)                                           # [128, 1152]
    w1_h = np.concatenate(
        [W1[g][j * 128:(j + 1) * 128] for g in range(3) for j in range(6)],
        axis=1).astype(BF16)                                           # [128, 2304]
    b0_h = np.stack(bias0, axis=1).astype(np.float32)                  # [128, 3]
    b1_h = np.stack(bias1, axis=1).astype(np.float32)

    wd1_h = np.asarray(dec["W1"], np.float32).astype(BF16)
    wd2_h = np.asarray(dec["W2"], np.float32).astype(BF16)
    bd1_h = np.asarray(dec["b1"], np.float32).reshape(H, 1)
    bd2_h = np.asarray(dec["b2"], np.float32).reshape(HORIZON, 1)

    in_maps = []
    for c in range(NCORES):
        fb = feats[c * BL:(c + 1) * BL]                  # [BL, t, N, 6]
        xd_h = np.ascontiguousarray(
            fb.transpose(1, 3, 0, 2).reshape(t_steps, 6, COLS)).astype(BF16)
        in_maps.append(dict(
            xd=xd_h, lt=lt_h, l2t=l2t_h, w0x=w0x_h, w0h=w0h_h, w1=w1_h,
            b0=b0_h, b1=b1_h, wd1=wd1_h, wd2=wd2_h, bd1=bd1_h, bd2=bd2_h))
    return in_maps


PROFILE = False          # set by test harness to capture an NTFF trace
LAST_RESULTS = None      # BassKernelResults of the last run (for profiling)


def kernel(x, L, params, dec):
    global LAST_RESULTS
    from concourse import bass_utils

    t_steps = T
    if t_steps not in _BUILD_CACHE:
        _BUILD_CACHE[t_steps] = _build(t_steps)
    nc = _BUILD_CACHE[t_steps]

    in_maps = _host_prep(x, L, params, dec, t_steps)
    res = bass_utils.run_bass_kernel_spmd(nc, in_maps,
                                          core_ids=list(range(NCORES)),
                                          trace=PROFILE)
    LAST_RESULTS = res
    out = np.empty((B, N, HORIZON), dtype=np.float32)
    for c in range(NCORES):
        yc = res.results[c]["y"]                         # [12, BL*N]
        out[c * BL:(c + 1) * BL] = (
            yc.reshape(HORIZON, BL, N).transpose(1, 2, 0))
    return out


# revision 9
# speedup vs baseline: 2.0233x; 2.0233x over previous
"""DCRNN Trainium2 kernel.

Model (see problem): B=128, T=24, N=512 nodes, F_IN=2, H=128, K=2 diffusion
steps, 2 stacked DCGRU layers, decoder MLP to HORIZON=12.

Strategy:
  - Data-parallel over batch: 8 cores x 16 local batch (BL).
  - All state stays in SBUF across the 24 recurrent steps (no HBM in loop).
  - Recurrent state h lives in "fm2" layout: [128 feat-part, b*512 + node].
    Gate GEMMs contract features -> read fm2 slices directly as matmul rhs.
    Diffusion contracts nodes -> per-b node-major operand produced by a
    single DMA-xbar transpose [128,512] -> [128,4,128] (off-engine).
  - L^2 is precomputed on host (L2 = L@L), so each diffusion trio is two
    accumulation groups (rhs = L^T and (L^2)^T) producing fm2 outputs.
  - x_diff (input diffusion features) precomputed on host (0.8% of FLOPs),
    fed per-step as a K=6 fm2 block; gate weights packed to match.
  - bf16 matmul operands, fp32 PSUM accumulation, fp32 biases.
"""

import numpy as np
import ml_dtypes
from contextlib import ExitStack

B, T, N, F_IN = 128, 24, 512, 2
H, KDIFF, LAYERS, HORIZON = 128, 2, 2, 12
NCORES = 8
BL = B // NCORES          # 16 local batch per core
NT = N // 128             # 4 node tiles
COLS = BL * N             # 8192 fm2 columns (b*512 + n)
BF16 = ml_dtypes.bfloat16

_BUILD_CACHE = {}


def _build(t_steps=T):
    import concourse.mybir as mybir
    import concourse.tile as tile
    from concourse import bacc

    f32 = mybir.dt.float32
    bf16 = mybir.dt.bfloat16
    AF = mybir.ActivationFunctionType

    nc = bacc.Bacc("TRN2", target_bir_lowering=False, debug=False,
                   enable_asserts=False, num_devices=NCORES)

    # ---- DRAM I/O ----
    xd = nc.dram_tensor("xd", [t_steps, 6, COLS], bf16, kind="ExternalInput").ap()
    lt = nc.dram_tensor("lt", [128, NT * N], bf16, kind="ExternalInput").ap()
    l2t = nc.dram_tensor("l2t", [128, NT * N], bf16, kind="ExternalInput").ap()
    w0x = nc.dram_tensor("w0x", [6, 3 * H], bf16, kind="ExternalInput").ap()
    w0h = nc.dram_tensor("w0h", [128, 9 * H], bf16, kind="ExternalInput").ap()
    w1 = nc.dram_tensor("w1", [128, 18 * H], bf16, kind="ExternalInput").ap()
    b0 = nc.dram_tensor("b0", [128, 3], f32, kind="ExternalInput").ap()
    b1 = nc.dram_tensor("b1", [128, 3], f32, kind="ExternalInput").ap()
    wd1 = nc.dram_tensor("wd1", [128, H], bf16, kind="ExternalInput").ap()
    wd2 = nc.dram_tensor("wd2", [128, HORIZON], bf16, kind="ExternalInput").ap()
    bd1 = nc.dram_tensor("bd1", [128, 1], f32, kind="ExternalInput").ap()
    bd2 = nc.dram_tensor("bd2", [HORIZON, 1], f32, kind="ExternalInput").ap()
    y = nc.dram_tensor("y", [HORIZON, COLS], f32, kind="ExternalOutput").ap()

    with tile.TileContext(nc) as tc, ExitStack() as ctx:
        consts = ctx.enter_context(tc.tile_pool(name="consts", bufs=1))
        state = ctx.enter_context(tc.tile_pool(name="state", bufs=2))
        work = ctx.enter_context(tc.tile_pool(name="work", bufs=4))
        xkp = ctx.enter_context(tc.tile_pool(name="xkp", bufs=2))
        pst = ctx.enter_context(tc.tile_pool(name="pst", bufs=4, space="PSUM"))
        psg = ctx.enter_context(tc.tile_pool(name="psg", bufs=4, space="PSUM"))

        # ---- load constants ----
        lt_sb = consts.tile([128, NT * N], bf16)
        nc.sync.dma_start(lt_sb, lt)
        l2t_sb = consts.tile([128, NT * N], bf16)
        nc.sync.dma_start(l2t_sb, l2t)
        w0x_sb = consts.tile([6, 3 * H], bf16)
        nc.sync.dma_start(w0x_sb, w0x)
        w0h_sb = consts.tile([128, 9 * H], bf16)
        nc.sync.dma_start(w0h_sb, w0h)
        w1_sb = consts.tile([128, 18 * H], bf16)
        nc.sync.dma_start(w1_sb, w1)
        b0_sb = consts.tile([128, 3], f32)
        nc.sync.dma_start(b0_sb, b0)
        b1_sb = consts.tile([128, 3], f32)
        nc.sync.dma_start(b1_sb, b1)
        wd1_sb = consts.tile([128, H], bf16)
        nc.sync.dma_start(wd1_sb, wd1)
        wd2_sb = consts.tile([128, HORIZON], bf16)
        nc.sync.dma_start(wd2_sb, wd2)
        bd1_sb = consts.tile([128, 1], f32)
        nc.sync.dma_start(bd1_sb, bd1)
        bd2_sb = consts.tile([HORIZON, 1], f32)
        nc.sync.dma_start(bd2_sb, bd2)

        # ---- recurrent state, fm2 layout [128 feat, b*512 + n] ----
        h0f = state.tile([128, COLS], bf16, tag="h0f")
        h1f = state.tile([128, COLS], bf16, tag="h1f")
        nc.vector.memset(h0f, 0.0)
        nc.vector.memset(h1f, 0.0)

        def bs(hf, b):
            return hf[:, b * N:(b + 1) * N]

        def to_nmb(src_slice, tag):
            """One DMA-xbar transpose: fm2 [128,512] -> nmb [128, 4*128]."""
            z = work.tile([128, N], bf16, tag=tag, name=tag)
            nc.sync.dma_start_transpose(
                z.rearrange("p (j f) -> p j f", j=NT), src_slice)
            return z

        def trio(nmbt, tag):
            """Diffuse: returns (L z, L^2 z) fm2 bf16 [128,512] tiles."""
            outs = []
            for mat, sfx, ev in ((lt_sb, "a", "v"), (l2t_sb, "b", "s")):
                ps = pst.tile([128, N], f32, tag="pst", name=f"ps_{tag}{sfx}")
                for mt in range(NT):
                    nc.tensor.matmul(ps, nmbt[:, mt * 128:(mt + 1) * 128],
                                     mat[:, mt * N:(mt + 1) * N],
                                     start=(mt == 0), stop=(mt == NT - 1))
                z = work.tile([128, N], bf16, tag=f"{tag}{sfx}",
                              name=f"{tag}{sfx}")
                if ev == "v":
                    nc.vector.tensor_copy(z, ps)
                else:
                    nc.scalar.copy(z, ps)
                outs.append(z)
            return outs

        def gate(feats, bias_ap, func, tag):
            """feats: list of (lhsT_w_ap, rhs_feat_ap). Returns bf16 tile."""
            ps = psg.tile([128, N], f32, tag="psg", name=f"ps_{tag}")
            nfe = len(feats)
            for i, (wap, fap) in enumerate(feats):
                nc.tensor.matmul(ps, wap, fap,
                                 start=(i == 0), stop=(i == nfe - 1))
            g = work.tile([128, N], bf16, tag=tag, name=tag)
            nc.scalar.activation(g, ps, func, bias=bias_ap)
            return g

        AFSig, AFTanh = AF.Sigmoid, AF.Tanh

        for t in range(t_steps):
            xk = xkp.tile([6, COLS], bf16, tag="xk", name="xk")
            nc.sync.dma_start(xk, xd[t])

            h0f_new = state.tile([128, COLS], bf16, tag="h0f", name="h0f_new")
            h1f_new = state.tile([128, COLS], bf16, tag="h1f", name="h1f_new")

            for layer in range(LAYERS):
                hf = h0f if layer == 0 else h1f
                hf_new = h0f_new if layer == 0 else h1f_new
                wsb = w0h_sb if layer == 0 else w1_sb
                bsb = b0_sb if layer == 0 else b1_sb

                for b in range(BL):
                    hslc = bs(hf, b)
                    hnb = to_nmb(hslc, "hnb")
                    lh, l2h = trio(hnb, "lh")
                    if layer == 0:
                        xfeat = [(w0x_sb[:, g * H:(g + 1) * H],
                                  xk[:, b * N:(b + 1) * N]) for g in range(3)]
                        hblk = lambda g, j: wsb[:, (g * 3 + j) * H:(g * 3 + j + 1) * H]
                        r = gate([xfeat[0],
                                  (hblk(0, 0), hslc), (hblk(0, 1), lh),
                                  (hblk(0, 2), l2h)],
                                 bsb[:, 0:1], AFSig, "r")
                        u = gate([xfeat[1],
                                  (hblk(1, 0), hslc), (hblk(1, 1), lh),
                                  (hblk(1, 2), l2h)],
                                 bsb[:, 1:2], AFSig, "u")
                    else:
                        xslc = bs(h0f_new, b)
                        xnb = to_nmb(xslc, "xnb")
                        lx, l2x = trio(xnb, "lx")
                        hblk = lambda g, j: wsb[:, (g * 6 + j) * H:(g * 6 + j + 1) * H]
                        r = gate([(hblk(0, 0), xslc), (hblk(0, 1), lx),
                                  (hblk(0, 2), l2x), (hblk(0, 3), hslc),
                                  (hblk(0, 4), lh), (hblk(0, 5), l2h)],
                                 bsb[:, 0:1], AFSig, "r")
                        u = gate([(hblk(1, 0), xslc), (hblk(1, 1), lx),
                                  (hblk(1, 2), l2x), (hblk(1, 3), hslc),
                                  (hblk(1, 4), lh), (hblk(1, 5), l2h)],
                                 bsb[:, 1:2], AFSig, "u")

                    rh = work.tile([128, N], bf16, tag="rh", name="rh")
                    nc.vector.tensor_mul(rh, r, hslc)
                    rhn = to_nmb(rh, "rhn")
                    lrh, l2rh = trio(rhn, "lr")
                    if layer == 0:
                        c = gate([xfeat[2],
                                  (hblk(2, 0), rh), (hblk(2, 1), lrh),
                                  (hblk(2, 2), l2rh)],
                                 bsb[:, 2:3], AFTanh, "c")
                    else:
                        c = gate([(hblk(2, 0), xslc), (hblk(2, 1), lx),
                                  (hblk(2, 2), l2x), (hblk(2, 3), rh),
                                  (hblk(2, 4), lrh), (hblk(2, 5), l2rh)],
                                 bsb[:, 2:3], AFTanh, "c")

                    # h' = h + u*(c-h), written straight into the new state
                    d = work.tile([128, N], bf16, tag="d", name="d")
                    nc.vector.tensor_sub(d, c, hslc)
                    e = work.tile([128, N], bf16, tag="e", name="e")
                    nc.vector.tensor_mul(e, u, d)
                    nc.gpsimd.tensor_add(bs(hf_new, b), hslc, e)

            h0f, h1f = h0f_new, h1f_new

        # ---- decoder ----
        for b in range(BL):
            ps = psg.tile([128, N], f32, tag="psg", name="ps_dec1")
            nc.tensor.matmul(ps, wd1_sb, bs(h1f, b), start=True, stop=True)
            hid = work.tile([128, N], bf16, tag="hid", name="hid")
            nc.scalar.activation(hid, ps, AF.Relu, bias=bd1_sb[:, 0:1])
            ps2 = psg.tile([HORIZON, N], f32, tag="psg", name="ps_dec2")
            nc.tensor.matmul(ps2, wd2_sb, hid, start=True, stop=True)
            yo = work.tile([HORIZON, N], f32, tag="yo", name="yo")
            nc.scalar.activation(yo, ps2, AF.Identity, bias=bd2_sb[:, 0:1])
            nc.sync.dma_start(y[:, b * N:(b + 1) * N], yo)

    nc.compile()
    return nc


def _host_prep(x, L, params, dec, t_steps=T):
    """Numpy preprocessing -> per-core in_maps."""
    x = np.asarray(x, dtype=np.float32)
    L = np.asarray(L, dtype=np.float32)
    Ld = L.astype(np.float64)
    L2 = (Ld @ Ld).astype(np.float32)

    lt_h = np.concatenate([L.T[mt * 128:(mt + 1) * 128] for mt in range(NT)],
                          axis=1).astype(BF16)           # [128, 4*512]
    l2t_h = np.concatenate([L2.T[mt * 128:(mt + 1) * 128] for mt in range(NT)],
                           axis=1).astype(BF16)

    # x_diff on host: feats [B, t, N, 6] = [x, Lx, L2x]
    xt = x[:, :t_steps]                                  # [B, t, N, 2]
    xp = xt.transpose(2, 0, 1, 3).reshape(N, -1)         # [N, B*t*2]
    lx = (L @ xp).reshape(N, B, t_steps, 2).transpose(1, 2, 0, 3)
    l2x = (L2 @ xp).reshape(N, B, t_steps, 2).transpose(1, 2, 0, 3)
    feats = np.concatenate([xt, lx, l2x], axis=-1)       # [B, t, N, 6]

    W0 = [np.asarray(params[0][k], np.float32) for k in ("Wr", "Wu", "Wc")]
    bias0 = [np.asarray(params[0][k], np.float32) for k in ("br", "bu", "bc")]
    W1 = [np.asarray(params[1][k], np.float32) for k in ("Wr", "Wu", "Wc")]
    bias1 = [np.asarray(params[1][k], np.float32) for k in ("br", "bu", "bc")]

    w0x_h = np.concatenate([W[0:6] for W in W0], axis=1).astype(BF16)  # [6, 384]
    w0h_h = np.concatenate(
        [W0[g][6 + j * 128: 6 + (j + 1) * 128] for g in range(3) for j in range(3)],
        axis=1).astype(BF16)                                           # [128, 1152]
    w1_h = np.concatenate(
        [W1[g][j * 128:(j + 1) * 128] for g in range(3) for j in range(6)],
        axis=1).astype(BF16)                                           # [128, 2304]
    b0_h = np.stack(bias0, axis=1).astype(np.float32)                  # [128, 3]
    b1_h = np.stack(bias1, axis=1).astype(np.float32)

    wd1_h = np.asarray(dec["W1"], np.float32).astype(BF16)
    wd2_h = np.asarray(dec["W2"], np.float32).astype(BF16)
    bd1_h = np.asarray(dec["b1"], np.float32).reshape(H, 1)
    bd2_h = np.asarray(dec["b2"], np.float32).reshape(HORIZON, 1)

    in_maps = []
    for c in range(NCORES):
        fb = feats[c * BL:(c + 1) * BL]                  # [BL, t, N, 6]
        xd_h = np.ascontiguousarray(
            fb.transpose(1, 3, 0, 2).reshape(t_steps, 6, COLS)).astype(BF16)
        in_maps.append(dict(
            xd=xd_h, lt=lt_h, l2t=l2t_h, w0x=w0x_h, w0h=w0h_h, w1=w1_h,
            b0=b0_h, b1=b1_h, wd1=wd1_h, wd2=wd2_h, bd1=bd1_h, bd2=bd2_h))
    return in_maps


PROFILE = False          # set by test harness to capture an NTFF trace
LAST_RESULTS = None      # BassKernelResults of the last run (for profiling)


def kernel(x, L, params, dec):
    global LAST_RESULTS
    from concourse import bass_utils

    t_steps = T
    if t_steps not in _BUILD_CACHE:
        _BUILD_CACHE[t_steps] = _build(t_steps)
    nc = _BUILD_CACHE[t_steps]

    in_maps = _host_prep(x, L, params, dec, t_steps)
    res = bass_utils.run_bass_kernel_spmd(nc, in_maps,
                                          core_ids=list(range(NCORES)),
                                          trace=PROFILE)
    LAST_RESULTS = res
    out = np.empty((B, N, HORIZON), dtype=np.float32)
    for c in range(NCORES):
        yc = res.results[c]["y"]                         # [12, BL*N]
        out[c * BL:(c + 1) * BL] = (
            yc.reshape(HORIZON, BL, N).transpose(1, 2, 0))
    return out


# revision 15
# speedup vs baseline: 2.0368x; 1.0066x over previous
"""DCRNN Trainium2 kernel.

Model (see problem): B=128, T=24, N=512 nodes, F_IN=2, H=128, K=2 diffusion
steps, 2 stacked DCGRU layers, decoder MLP to HORIZON=12.

Strategy:
  - Data-parallel over batch: 8 cores x 16 local batch (BL).
  - All state stays in SBUF across the 24 recurrent steps (no HBM in loop).
  - Recurrent state h lives in "fm2" layout: [128 feat-part, b*512 + node].
    Gate GEMMs contract features -> read fm2 slices directly as matmul rhs.
    Diffusion contracts nodes -> per-b node-major operand produced by a
    single DMA-xbar transpose [128,512] -> [128,4,128] (off-engine).
  - L^2 is precomputed on host (L2 = L@L), so each diffusion trio is two
    accumulation groups (rhs = L^T and (L^2)^T) producing fm2 outputs.
  - x_diff (input diffusion features) precomputed on host (0.8% of FLOPs),
    fed per-step as a K=6 fm2 block; gate weights packed to match.
  - bf16 matmul operands, fp32 PSUM accumulation, fp32 biases.
"""

import numpy as np
import ml_dtypes
from contextlib import ExitStack

B, T, N, F_IN = 128, 24, 512, 2
H, KDIFF, LAYERS, HORIZON = 128, 2, 2, 12
NCORES = 8
BL = B // NCORES          # 16 local batch per core
NT = N // 128             # 4 node tiles
COLS = BL * N             # 8192 fm2 columns (b*512 + n)
BF16 = ml_dtypes.bfloat16

_BUILD_CACHE = {}


def _build(t_steps=T):
    import concourse.mybir as mybir
    import concourse.tile as tile
    from concourse import bacc

    f32 = mybir.dt.float32
    bf16 = mybir.dt.bfloat16
    AF = mybir.ActivationFunctionType

    nc = bacc.Bacc("TRN2", target_bir_lowering=False, debug=False,
                   enable_asserts=False, num_devices=NCORES)

    # ---- DRAM I/O ----
    xd = nc.dram_tensor("xd", [t_steps, 6, COLS], bf16, kind="ExternalInput").ap()
    lt = nc.dram_tensor("lt", [128, NT * N], bf16, kind="ExternalInput").ap()
    l2t = nc.dram_tensor("l2t", [128, NT * N], bf16, kind="ExternalInput").ap()
    w0x = nc.dram_tensor("w0x", [6, 3 * H], bf16, kind="ExternalInput").ap()
    w0h = nc.dram_tensor("w0h", [128, 9 * H], bf16, kind="ExternalInput").ap()
    w1 = nc.dram_tensor("w1", [128, 18 * H], bf16, kind="ExternalInput").ap()
    b0 = nc.dram_tensor("b0", [128, 3], f32, kind="ExternalInput").ap()
    b1 = nc.dram_tensor("b1", [128, 3], f32, kind="ExternalInput").ap()
    wd1 = nc.dram_tensor("wd1", [128, H], bf16, kind="ExternalInput").ap()
    wd2 = nc.dram_tensor("wd2", [128, HORIZON], bf16, kind="ExternalInput").ap()
    bd1 = nc.dram_tensor("bd1", [128, 1], f32, kind="ExternalInput").ap()
    bd2 = nc.dram_tensor("bd2", [HORIZON, 1], f32, kind="ExternalInput").ap()
    y = nc.dram_tensor("y", [HORIZON, COLS], f32, kind="ExternalOutput").ap()

    with tile.TileContext(nc) as tc, ExitStack() as ctx:
        consts = ctx.enter_context(tc.tile_pool(name="consts", bufs=1))
        state = ctx.enter_context(tc.tile_pool(name="state", bufs=2))
        work = ctx.enter_context(tc.tile_pool(name="work", bufs=4))
        xkp = ctx.enter_context(tc.tile_pool(name="xkp", bufs=2))
        pst = ctx.enter_context(tc.tile_pool(name="pst", bufs=5, space="PSUM"))
        psg = ctx.enter_context(tc.tile_pool(name="psg", bufs=3, space="PSUM"))

        # ---- load constants ----
        lt_sb = consts.tile([128, NT * N], bf16)
        nc.sync.dma_start(lt_sb, lt)
        l2t_sb = consts.tile([128, NT * N], bf16)
        nc.sync.dma_start(l2t_sb, l2t)
        w0x_sb = consts.tile([6, 3 * H], bf16)
        nc.sync.dma_start(w0x_sb, w0x)
        w0h_sb = consts.tile([128, 9 * H], bf16)
        nc.sync.dma_start(w0h_sb, w0h)
        w1_sb = consts.tile([128, 18 * H], bf16)
        nc.sync.dma_start(w1_sb, w1)
        b0_sb = consts.tile([128, 3], f32)
        nc.sync.dma_start(b0_sb, b0)
        b1_sb = consts.tile([128, 3], f32)
        nc.sync.dma_start(b1_sb, b1)
        wd1_sb = consts.tile([128, H], bf16)
        nc.sync.dma_start(wd1_sb, wd1)
        wd2_sb = consts.tile([128, HORIZON], bf16)
        nc.sync.dma_start(wd2_sb, wd2)
        bd1_sb = consts.tile([128, 1], f32)
        nc.sync.dma_start(bd1_sb, bd1)
        bd2_sb = consts.tile([HORIZON, 1], f32)
        nc.sync.dma_start(bd2_sb, bd2)

        # ---- recurrent state, fm2 layout [128 feat, b*512 + n] ----
        h0f = state.tile([128, COLS], bf16, tag="h0f")
        h1f = state.tile([128, COLS], bf16, tag="h1f")
        nc.vector.memset(h0f, 0.0)
        nc.vector.memset(h1f, 0.0)

        def bs(hf, b):
            return hf[:, b * N:(b + 1) * N]

        def to_nmb(src_slice, tag):
            """One DMA-xbar transpose: fm2 [128,512] -> nmb [128, 4*128]."""
            z = work.tile([128, N], bf16, tag=tag, name=tag, bufs=6)
            nc.sync.dma_start_transpose(
                z.rearrange("p (j f) -> p j f", j=NT), src_slice)
            return z

        def trio(nmbt, tag):
            """Diffuse: returns (L z, L^2 z) fm2 bf16 [128,512] tiles.
            The two accumulation chains are interleaved per m-tile so
            consecutive matmuls share the stationary operand."""
            psa = pst.tile([128, N], f32, tag="pst", name=f"ps_{tag}a")
            psb = pst.tile([128, N], f32, tag="pst", name=f"ps_{tag}b")
            for mt in range(NT):
                blk = nmbt[:, mt * 128:(mt + 1) * 128]
                nc.tensor.matmul(psa, blk, lt_sb[:, mt * N:(mt + 1) * N],
                                 start=(mt == 0), stop=(mt == NT - 1))
                nc.tensor.matmul(psb, blk, l2t_sb[:, mt * N:(mt + 1) * N],
                                 start=(mt == 0), stop=(mt == NT - 1))
            za = work.tile([128, N], bf16, tag=f"{tag}a", name=f"{tag}a")
            nc.vector.tensor_copy(za, psa)
            zb = work.tile([128, N], bf16, tag=f"{tag}b", name=f"{tag}b")
            nc.scalar.copy(zb, psb)
            return za, zb

        def gate(feats, bias_ap, func, tag):
            """feats: list of (lhsT_w_ap, rhs_feat_ap). Returns bf16 tile."""
            ps = psg.tile([128, N], f32, tag="psg", name=f"ps_{tag}")
            nfe = len(feats)
            for i, (wap, fap) in enumerate(feats):
                nc.tensor.matmul(ps, wap, fap,
                                 start=(i == 0), stop=(i == nfe - 1))
            g = work.tile([128, N], bf16, tag=tag, name=tag)
            nc.scalar.activation(g, ps, func, bias=bias_ap)
            return g

        AFSig, AFTanh = AF.Sigmoid, AF.Tanh

        for t in range(t_steps):
            xk = xkp.tile([6, COLS], bf16, tag="xk", name="xk")
            nc.gpsimd.dma_start(xk, xd[t])

            h0f_new = state.tile([128, COLS], bf16, tag="h0f", name="h0f_new")
            h1f_new = state.tile([128, COLS], bf16, tag="h1f", name="h1f_new")

            for layer in range(LAYERS):
                hf = h0f if layer == 0 else h1f
                hf_new = h0f_new if layer == 0 else h1f_new
                wsb = w0h_sb if layer == 0 else w1_sb
                bsb = b0_sb if layer == 0 else b1_sb

                for b in range(BL):
                    hslc = bs(hf, b)
                    hnb = to_nmb(hslc, "hnb")
                    lh, l2h = trio(hnb, "lh")
                    if layer == 0:
                        xfeat = [(w0x_sb[:, g * H:(g + 1) * H],
                                  xk[:, b * N:(b + 1) * N]) for g in range(3)]
                        hblk = lambda g, j: wsb[:, (g * 3 + j) * H:(g * 3 + j + 1) * H]
                        r = gate([xfeat[0],
                                  (hblk(0, 0), hslc), (hblk(0, 1), lh),
                                  (hblk(0, 2), l2h)],
                                 bsb[:, 0:1], AFSig, "r")
                        u = gate([xfeat[1],
                                  (hblk(1, 0), hslc), (hblk(1, 1), lh),
                                  (hblk(1, 2), l2h)],
                                 bsb[:, 1:2], AFSig, "u")
                    else:
                        xslc = bs(h0f_new, b)
                        xnb = to_nmb(xslc, "xnb")
                        lx, l2x = trio(xnb, "lx")
                        hblk = lambda g, j: wsb[:, (g * 6 + j) * H:(g * 6 + j + 1) * H]
                        r = gate([(hblk(0, 0), xslc), (hblk(0, 1), lx),
                                  (hblk(0, 2), l2x), (hblk(0, 3), hslc),
                                  (hblk(0, 4), lh), (hblk(0, 5), l2h)],
                                 bsb[:, 0:1], AFSig, "r")
                        u = gate([(hblk(1, 0), xslc), (hblk(1, 1), lx),
                                  (hblk(1, 2), l2x), (hblk(1, 3), hslc),
                                  (hblk(1, 4), lh), (hblk(1, 5), l2h)],
                                 bsb[:, 1:2], AFSig, "u")

                    rh = work.tile([128, N], bf16, tag="rh", name="rh")
                    nc.vector.tensor_mul(rh, r, hslc)
                    rhn = to_nmb(rh, "rhn")
                    lrh, l2rh = trio(rhn, "lr")
                    if layer == 0:
                        c = gate([xfeat[2],
                                  (hblk(2, 0), rh), (hblk(2, 1), lrh),
                                  (hblk(2, 2), l2rh)],
                                 bsb[:, 2:3], AFTanh, "c")
                    else:
                        c = gate([(hblk(2, 0), xslc), (hblk(2, 1), lx),
                                  (hblk(2, 2), l2x), (hblk(2, 3), rh),
                                  (hblk(2, 4), lrh), (hblk(2, 5), l2rh)],
                                 bsb[:, 2:3], AFTanh, "c")

                    # h' = h + u*(c-h), written straight into the new state
                    d = work.tile([128, N], bf16, tag="d", name="d")
                    nc.vector.tensor_sub(d, c, hslc)
                    e = work.tile([128, N], bf16, tag="e", name="e")
                    nc.vector.tensor_mul(e, u, d)
                    nc.gpsimd.tensor_add(bs(hf_new, b), hslc, e)

            h0f, h1f = h0f_new, h1f_new

        # ---- decoder ----
        for b in range(BL):
            ps = psg.tile([128, N], f32, tag="psg", name="ps_dec1")
            nc.tensor.matmul(ps, wd1_sb, bs(h1f, b), start=True, stop=True)
            hid = work.tile([128, N], bf16, tag="hid", name="hid")
            nc.scalar.activation(hid, ps, AF.Relu, bias=bd1_sb[:, 0:1])
            ps2 = psg.tile([HORIZON, N], f32, tag="psg", name="ps_dec2")
            nc.tensor.matmul(ps2, wd2_sb, hid, start=True, stop=True)
            yo = work.tile([HORIZON, N], f32, tag="yo", name="yo")
            nc.scalar.activation(yo, ps2, AF.Identity, bias=bd2_sb[:, 0:1])
            nc.gpsimd.dma_start(y[:, b * N:(b + 1) * N], yo)

    nc.compile()
    return nc


def _host_prep(x, L, params, dec, t_steps=T):
    """Numpy preprocessing -> per-core in_maps."""
    x = np.asarray(x, dtype=np.float32)
    L = np.asarray(L, dtype=np.float32)
    Ld = L.astype(np.float64)
    L2 = (Ld @ Ld).astype(np.float32)

    lt_h = np.concatenate([L.T[mt * 128:(mt + 1) * 128] for mt in range(NT)],
                          axis=1).astype(BF16)           # [128, 4*512]
    l2t_h = np.concatenate([L2.T[mt * 128:(mt + 1) * 128] for mt in range(NT)],
                           axis=1).astype(BF16)

    # x_diff on host: feats [B, t, N, 6] = [x, Lx, L2x]
    xt = x[:, :t_steps]                                  # [B, t, N, 2]
    xp = xt.transpose(2, 0, 1, 3).reshape(N, -1)         # [N, B*t*2]
    lx = (L @ xp).reshape(N, B, t_steps, 2).transpose(1, 2, 0, 3)
    l2x = (L2 @ xp).reshape(N, B, t_steps, 2).transpose(1, 2, 0, 3)
    feats = np.concatenate([xt, lx, l2x], axis=-1)       # [B, t, N, 6]

    W0 = [np.asarray(params[0][k], np.float32) for k in ("Wr", "Wu", "Wc")]
    bias0 = [np.asarray(params[0][k], np.float32) for k in ("br", "bu", "bc")]
    W1 = [np.asarray(params[1][k], np.float32) for k in ("Wr", "Wu", "Wc")]
    bias1 = [np.asarray(params[1][k], np.float32) for k in ("br", "bu", "bc")]

    w0x_h = np.concatenate([W[0:6] for W in W0], axis=1).astype(BF16)  # [6, 384]
    w0h_h = np.concatenate(
        [W0[g][6 + j * 128: 6 + (j + 1) * 128] for g in range(3) for j in range(3)],
        axis=1).astype(BF16)                                           # [128, 1152]
    w1_h = np.concatenate(
        [W1[g][j * 128:(j + 1) * 128] for g in range(3) for j in range(6)],
        axis=1).astype(BF16)                                           # [128, 2304]
    b0_h = np.stack(bias0, axis=1).astype(np.float32)                  # [128, 3]
    b1_h = np.stack(bias1, axis=1).astype(np.float32)

    wd1_h = np.asarray(dec["W1"], np.float32).astype(BF16)
    wd2_h = np.asarray(dec["W2"], np.float32).astype(BF16)
    bd1_h = np.asarray(dec["b1"], np.float32).reshape(H, 1)
    bd2_h = np.asarray(dec["b2"], np.float32).reshape(HORIZON, 1)

    in_maps = []
    for c in range(NCORES):
        fb = feats[c * BL:(c + 1) * BL]                  # [BL, t, N, 6]
        xd_h = np.ascontiguousarray(
            fb.transpose(1, 3, 0, 2).reshape(t_steps, 6, COLS)).astype(BF16)
        in_maps.append(dict(
            xd=xd_h, lt=lt_h, l2t=l2t_h, w0x=w0x_h, w0h=w0h_h, w1=w1_h,
            b0=b0_h, b1=b1_h, wd1=wd1_h, wd2=wd2_h, bd1=bd1_h, bd2=bd2_h))
    return in_maps


PROFILE = False          # set by test harness to capture an NTFF trace
LAST_RESULTS = None      # BassKernelResults of the last run (for profiling)


def kernel(x, L, params, dec):
    global LAST_RESULTS
    from concourse import bass_utils

    t_steps = T
    if t_steps not in _BUILD_CACHE:
        _BUILD_CACHE[t_steps] = _build(t_steps)
    nc = _BUILD_CACHE[t_steps]

    in_maps = _host_prep(x, L, params, dec, t_steps)
    res = bass_utils.run_bass_kernel_spmd(nc, in_maps,
                                          core_ids=list(range(NCORES)),
                                          trace=PROFILE)
    LAST_RESULTS = res
    out = np.empty((B, N, HORIZON), dtype=np.float32)
    for c in range(NCORES):
        yc = res.results[c]["y"]                         # [12, BL*N]
        out[c * BL:(c + 1) * BL] = (
            yc.reshape(HORIZON, BL, N).transpose(1, 2, 0))
    return out


# revision 19
# speedup vs baseline: 2.6135x; 1.2832x over previous
"""DCRNN Trainium2 kernel.

Model (see problem): B=128, T=24, N=512 nodes, F_IN=2, H=128, K=2 diffusion
steps, 2 stacked DCGRU layers, decoder MLP to HORIZON=12.

Strategy:
  - Data-parallel over batch: 8 cores x 16 local batch (BL).
  - All state stays in SBUF across the 24 recurrent steps (no HBM in loop).
  - Recurrent state h lives in "fm2" layout: [128 feat-part, b*512 + node],
    updated in place (subtile deps order readers/writer per b-slice).
    Gate GEMMs contract features -> read fm2 slices directly as matmul rhs.
    Diffusion contracts nodes -> node-major operands produced by DMA-xbar
    transposes batched over groups of 4 batch elements
    ([128, 2048] -> [128, 16, 128]) to keep the Sync dispatch queue short.
  - L^2 is precomputed on host (L2 = L@L), so each diffusion trio is two
    interleaved accumulation chains (rhs = L^T and (L^2)^T) into one
    2-bank PSUM tile, evicted with a single copy.
  - x_diff (input diffusion features) precomputed on host (0.8% of FLOPs),
    fed per-step as a K=6 fm2 block; gate weights packed to match.
  - bf16 matmul operands, fp32 PSUM accumulation, fp32 biases.
"""

import numpy as np
import ml_dtypes
from contextlib import ExitStack

B, T, N, F_IN = 128, 24, 512, 2
H, KDIFF, LAYERS, HORIZON = 128, 2, 2, 12
NCORES = 8
BL = B // NCORES          # 16 local batch per core
NT = N // 128             # 4 node tiles
COLS = BL * N             # 8192 fm2 columns (b*512 + n)
GRP = 4                   # batch group size for batched transposes
NGRP = BL // GRP
BF16 = ml_dtypes.bfloat16

_BUILD_CACHE = {}


def _build(t_steps=T):
    import concourse.mybir as mybir
    import concourse.tile as tile
    from concourse import bacc

    f32 = mybir.dt.float32
    bf16 = mybir.dt.bfloat16
    AF = mybir.ActivationFunctionType

    nc = bacc.Bacc("TRN2", target_bir_lowering=False, debug=False,
                   enable_asserts=False, num_devices=NCORES)

    # ---- DRAM I/O ----
    xd = nc.dram_tensor("xd", [t_steps, 6, COLS], bf16, kind="ExternalInput").ap()
    lt = nc.dram_tensor("lt", [128, NT * N], bf16, kind="ExternalInput").ap()
    l2t = nc.dram_tensor("l2t", [128, NT * N], bf16, kind="ExternalInput").ap()
    w0x = nc.dram_tensor("w0x", [6, 3 * H], bf16, kind="ExternalInput").ap()
    w0h = nc.dram_tensor("w0h", [128, 9 * H], bf16, kind="ExternalInput").ap()
    w1 = nc.dram_tensor("w1", [128, 18 * H], bf16, kind="ExternalInput").ap()
    b0 = nc.dram_tensor("b0", [128, 3], f32, kind="ExternalInput").ap()
    b1 = nc.dram_tensor("b1", [128, 3], f32, kind="ExternalInput").ap()
    wd1 = nc.dram_tensor("wd1", [128, H], bf16, kind="ExternalInput").ap()
    wd2 = nc.dram_tensor("wd2", [128, HORIZON], bf16, kind="ExternalInput").ap()
    bd1 = nc.dram_tensor("bd1", [128, 1], f32, kind="ExternalInput").ap()
    bd2 = nc.dram_tensor("bd2", [HORIZON, 1], f32, kind="ExternalInput").ap()
    y = nc.dram_tensor("y", [HORIZON, COLS], f32, kind="ExternalOutput").ap()

    with tile.TileContext(nc) as tc, ExitStack() as ctx:
        consts = ctx.enter_context(tc.tile_pool(name="consts", bufs=1))
        work = ctx.enter_context(tc.tile_pool(name="work", bufs=4))
        xkp = ctx.enter_context(tc.tile_pool(name="xkp", bufs=2))
        pst = ctx.enter_context(tc.tile_pool(name="pst", bufs=2, space="PSUM"))
        psg = ctx.enter_context(tc.tile_pool(name="psg", bufs=4, space="PSUM"))

        # ---- load constants ----
        lt_sb = consts.tile([128, NT * N], bf16)
        nc.sync.dma_start(lt_sb, lt)
        l2t_sb = consts.tile([128, NT * N], bf16)
        nc.sync.dma_start(l2t_sb, l2t)
        w0x_sb = consts.tile([6, 3 * H], bf16)
        nc.sync.dma_start(w0x_sb, w0x)
        w0h_sb = consts.tile([128, 9 * H], bf16)
        nc.sync.dma_start(w0h_sb, w0h)
        w1_sb = consts.tile([128, 18 * H], bf16)
        nc.sync.dma_start(w1_sb, w1)
        b0_sb = consts.tile([128, 3], f32)
        nc.sync.dma_start(b0_sb, b0)
        b1_sb = consts.tile([128, 3], f32)
        nc.sync.dma_start(b1_sb, b1)
        wd1_sb = consts.tile([128, H], bf16)
        nc.sync.dma_start(wd1_sb, wd1)
        wd2_sb = consts.tile([128, HORIZON], bf16)
        nc.sync.dma_start(wd2_sb, wd2)
        bd1_sb = consts.tile([128, 1], f32)
        nc.sync.dma_start(bd1_sb, bd1)
        bd2_sb = consts.tile([HORIZON, 1], f32)
        nc.sync.dma_start(bd2_sb, bd2)

        # ---- recurrent state, fm2 layout, updated in place ----
        h0f = consts.tile([128, COLS], bf16)
        h1f = consts.tile([128, COLS], bf16)
        nc.vector.memset(h0f, 0.0)
        nc.vector.memset(h1f, 0.0)

        def bs(hf, b):
            return hf[:, b * N:(b + 1) * N]

        def tp_group(src_cols, tag):
            """Batched DMA-xbar transpose: fm2 [128, GRP*512] ->
            [128, GRP*4, 128]; block (bl*4 + j) holds nodes j*128.. for
            group-local batch bl."""
            z = work.tile([128, GRP * N], bf16, tag=tag, name=tag, bufs=3)
            nc.sync.dma_start_transpose(
                z.rearrange("p (k f) -> p k f", f=128), src_cols)
            return z

        def trio(nmb4, bl, tag, ev):
            """Diffuse group-local batch bl of a [128, GRP*4*128] nmb tile.
            Returns a [128, 1024] bf16 tile: cols 0:512 = L z, 512:1024 = L^2 z."""
            ps = pst.tile([128, 2 * N], f32, tag="pst", name=f"ps_{tag}")
            psa, psb = ps[:, 0:N], ps[:, N:2 * N]
            for mt in range(NT):
                blk = nmb4[:, (bl * NT + mt) * 128:(bl * NT + mt + 1) * 128]
                nc.tensor.matmul(psa, blk, lt_sb[:, mt * N:(mt + 1) * N],
                                 start=(mt == 0), stop=(mt == NT - 1))
                nc.tensor.matmul(psb, blk, l2t_sb[:, mt * N:(mt + 1) * N],
                                 start=(mt == 0), stop=(mt == NT - 1))
            zab = work.tile([128, 2 * N], bf16, tag=f"{tag}ab", name=f"{tag}ab")
            if ev == "v":
                nc.vector.tensor_copy(zab, ps)
            else:
                nc.scalar.copy(zab, ps)
            return zab

        def trio_held(nmb4, bl, tag, ev):
            """Like trio() but with deeper buffering: output is held across
            the group's r/u phase into the c phase."""
            ps = pst.tile([128, 2 * N], f32, tag="pst", name=f"ps_{tag}")
            psa, psb = ps[:, 0:N], ps[:, N:2 * N]
            for mt in range(NT):
                blk = nmb4[:, (bl * NT + mt) * 128:(bl * NT + mt + 1) * 128]
                nc.tensor.matmul(psa, blk, lt_sb[:, mt * N:(mt + 1) * N],
                                 start=(mt == 0), stop=(mt == NT - 1))
                nc.tensor.matmul(psb, blk, l2t_sb[:, mt * N:(mt + 1) * N],
                                 start=(mt == 0), stop=(mt == NT - 1))
            zab = work.tile([128, 2 * N], bf16, tag=f"{tag}ab",
                            name=f"{tag}ab", bufs=6)
            if ev == "v":
                nc.vector.tensor_copy(zab, ps)
            else:
                nc.scalar.copy(zab, ps)
            return zab

        def gate(feats, bias_ap, func, tag):
            ps = psg.tile([128, N], f32, tag="psg", name=f"ps_{tag}")
            nfe = len(feats)
            for i, (wap, fap) in enumerate(feats):
                nc.tensor.matmul(ps, wap, fap,
                                 start=(i == 0), stop=(i == nfe - 1))
            g = work.tile([128, N], bf16, tag=tag, name=tag)
            nc.scalar.activation(g, ps, func, bias=bias_ap)
            return g

        AFSig, AFTanh = AF.Sigmoid, AF.Tanh

        for t in range(t_steps):
            xk = xkp.tile([6, COLS], bf16, tag="xk", name="xk")
            nc.gpsimd.dma_start(xk, xd[t])

            for layer in range(LAYERS):
                hf = h0f if layer == 0 else h1f
                wsb = w0h_sb if layer == 0 else w1_sb
                bsb = b0_sb if layer == 0 else b1_sb

                for g in range(NGRP):
                    gcols = slice(g * GRP * N, (g + 1) * GRP * N)
                    hnb4 = tp_group(hf[:, gcols], "hnb4")
                    if layer == 1:
                        xnb4 = tp_group(h0f[:, gcols], "xnb4")
                    rh4 = work.tile([128, GRP * N], bf16, tag="rh4",
                                    name="rh4", bufs=3)

                    gate_in = []   # per-bl list of (r_feats, u_feats, c_x_feats)
                    for bl in range(GRP):
                        b = g * GRP + bl
                        hslc = bs(hf, b)
                        lhab = trio(hnb4, bl, "lh", "v")
                        lh, l2h = lhab[:, 0:N], lhab[:, N:2 * N]
                        if layer == 0:
                            xf3 = [(w0x_sb[:, gg * H:(gg + 1) * H],
                                    xk[:, b * N:(b + 1) * N]) for gg in range(3)]
                            hb = lambda gg, j: wsb[:, (gg * 3 + j) * H:(gg * 3 + j + 1) * H]
                            rfe = [xf3[0], (hb(0, 0), hslc), (hb(0, 1), lh),
                                   (hb(0, 2), l2h)]
                            ufe = [xf3[1], (hb(1, 0), hslc), (hb(1, 1), lh),
                                   (hb(1, 2), l2h)]
                            cfe = [xf3[2]]
                        else:
                            xslc = bs(h0f, b)
                            lxab = trio_held(xnb4, bl, "lx", "v")
                            lx, l2x = lxab[:, 0:N], lxab[:, N:2 * N]
                            hb = lambda gg, j: wsb[:, (gg * 6 + j) * H:(gg * 6 + j + 1) * H]
                            rfe = [(hb(0, 0), xslc), (hb(0, 1), lx),
                                   (hb(0, 2), l2x), (hb(0, 3), hslc),
                                   (hb(0, 4), lh), (hb(0, 5), l2h)]
                            ufe = [(hb(1, 0), xslc), (hb(1, 1), lx),
                                   (hb(1, 2), l2x), (hb(1, 3), hslc),
                                   (hb(1, 4), lh), (hb(1, 5), l2h)]
                            cfe = [(hb(2, 0), xslc), (hb(2, 1), lx),
                                   (hb(2, 2), l2x)]

                        r = gate(rfe, bsb[:, 0:1], AFSig, "r")
                        ps_u = psg.tile([128, N], f32, tag="psg", name="ps_u")
                        for i, (wap, fap) in enumerate(ufe):
                            nc.tensor.matmul(ps_u, wap, fap, start=(i == 0),
                                             stop=(i == len(ufe) - 1))
                        u = work.tile([128, N], bf16, tag="u", name="u", bufs=6)
                        nc.scalar.activation(u, ps_u, AFSig, bias=bsb[:, 1:2])
                        nc.vector.tensor_mul(rh4[:, bl * N:(bl + 1) * N],
                                             r, hslc)
                        gate_in.append((u, cfe))

                    rhn4 = tp_group(rh4, "rhn4")

                    for bl in range(GRP):
                        b = g * GRP + bl
                        hslc = bs(hf, b)
                        u, cfe = gate_in[bl]
                        lrab = trio(rhn4, bl, "lr", "s")
                        lrh, l2rh = lrab[:, 0:N], lrab[:, N:2 * N]
                        rhs_slc = rh4[:, bl * N:(bl + 1) * N]
                        if layer == 0:
                            hb = lambda gg, j: wsb[:, (gg * 3 + j) * H:(gg * 3 + j + 1) * H]
                            cfeats = cfe + [(hb(2, 0), rhs_slc),
                                            (hb(2, 1), lrh), (hb(2, 2), l2rh)]
                        else:
                            hb = lambda gg, j: wsb[:, (gg * 6 + j) * H:(gg * 6 + j + 1) * H]
                            cfeats = cfe + [(hb(2, 3), rhs_slc),
                                            (hb(2, 4), lrh), (hb(2, 5), l2rh)]
                        c = gate(cfeats, bsb[:, 2:3], AFTanh, "c")

                        # h' = h + u*(c-h), in place
                        d = work.tile([128, N], bf16, tag="d", name="d")
                        nc.vector.tensor_sub(d, c, hslc)
                        e = work.tile([128, N], bf16, tag="e", name="e")
                        nc.vector.tensor_mul(e, u, d)
                        nc.gpsimd.tensor_add(hslc, hslc, e)

        # ---- decoder ----
        for b in range(BL):
            ps = psg.tile([128, N], f32, tag="psg", name="ps_dec1")
            nc.tensor.matmul(ps, wd1_sb, bs(h1f, b), start=True, stop=True)
            hid = work.tile([128, N], bf16, tag="hid", name="hid")
            nc.scalar.activation(hid, ps, AF.Relu, bias=bd1_sb[:, 0:1])
            ps2 = psg.tile([HORIZON, N], f32, tag="psg", name="ps_dec2")
            nc.tensor.matmul(ps2, wd2_sb, hid, start=True, stop=True)
            yo = work.tile([HORIZON, N], f32, tag="yo", name="yo")
            nc.scalar.activation(yo, ps2, AF.Identity, bias=bd2_sb[:, 0:1])
            nc.gpsimd.dma_start(y[:, b * N:(b + 1) * N], yo)

    nc.compile()
    return nc


def _host_prep(x, L, params, dec, t_steps=T):
    """Numpy preprocessing -> per-core in_maps."""
    x = np.asarray(x, dtype=np.float32)
    L = np.asarray(L, dtype=np.float32)
    Ld = L.astype(np.float64)
    L2 = (Ld @ Ld).astype(np.float32)

    lt_h = np.concatenate([L.T[mt * 128:(mt + 1) * 128] for mt in range(NT)],
                          axis=1).astype(BF16)           # [128, 4*512]
    l2t_h = np.concatenate([L2.T[mt * 128:(mt + 1) * 128] for mt in range(NT)],
                           axis=1).astype(BF16)

    # x_diff on host: feats [B, t, N, 6] = [x, Lx, L2x]
    xt = x[:, :t_steps]                                  # [B, t, N, 2]
    xp = xt.transpose(2, 0, 1, 3).reshape(N, -1)         # [N, B*t*2]
    lx = (L @ xp).reshape(N, B, t_steps, 2).transpose(1, 2, 0, 3)
    l2x = (L2 @ xp).reshape(N, B, t_steps, 2).transpose(1, 2, 0, 3)
    feats = np.concatenate([xt, lx, l2x], axis=-1)       # [B, t, N, 6]

    W0 = [np.asarray(params[0][k], np.float32) for k in ("Wr", "Wu", "Wc")]
    bias0 = [np.asarray(params[0][k], np.float32) for k in ("br", "bu", "bc")]
    W1 = [np.asarray(params[1][k], np.float32) for k in ("Wr", "Wu", "Wc")]
    bias1 = [np.asarray(params[1][k], np.float32) for k in ("br", "bu", "bc")]

    w0x_h = np.concatenate([W[0:6] for W in W0], axis=1).astype(BF16)  # [6, 384]
    w0h_h = np.concatenate(
        [W0[g][6 + j * 128: 6 + (j + 1) * 128] for g in range(3) for j in range(3)],
        axis=1).astype(BF16)                                           # [128, 1152]
    w1_h = np.concatenate(
        [W1[g][j * 128:(j + 1) * 128] for g in range(3) for j in range(6)],
        axis=1).astype(BF16)                                           # [128, 2304]
    b0_h = np.stack(bias0, axis=1).astype(np.float32)                  # [128, 3]
    b1_h = np.stack(bias1, axis=1).astype(np.float32)

    wd1_h = np.asarray(dec["W1"], np.float32).astype(BF16)
    wd2_h = np.asarray(dec["W2"], np.float32).astype(BF16)
    bd1_h = np.asarray(dec["b1"], np.float32).reshape(H, 1)
    bd2_h = np.asarray(dec["b2"], np.float32).reshape(HORIZON, 1)

    in_maps = []
    for c in range(NCORES):
        fb = feats[c * BL:(c + 1) * BL]                  # [BL, t, N, 6]
        xd_h = np.ascontiguousarray(
            fb.transpose(1, 3, 0, 2).reshape(t_steps, 6, COLS)).astype(BF16)
        in_maps.append(dict(
            xd=xd_h, lt=lt_h, l2t=l2t_h, w0x=w0x_h, w0h=w0h_h, w1=w1_h,
            b0=b0_h, b1=b1_h, wd1=wd1_h, wd2=wd2_h, bd1=bd1_h, bd2=bd2_h))
    return in_maps


PROFILE = False          # set by test harness to capture an NTFF trace
LAST_RESULTS = None      # BassKernelResults of the last run (for profiling)


def kernel(x, L, params, dec):
    global LAST_RESULTS
    from concourse import bass_utils

    t_steps = T
    if t_steps not in _BUILD_CACHE:
        _BUILD_CACHE[t_steps] = _build(t_steps)
    nc = _BUILD_CACHE[t_steps]

    in_maps = _host_prep(x, L, params, dec, t_steps)
    res = bass_utils.run_bass_kernel_spmd(nc, in_maps,
                                          core_ids=list(range(NCORES)),
                                          trace=PROFILE)
    LAST_RESULTS = res
    out = np.empty((B, N, HORIZON), dtype=np.float32)
    for c in range(NCORES):
        yc = res.results[c]["y"]                         # [12, BL*N]
        out[c * BL:(c + 1) * BL] = (
            yc.reshape(HORIZON, BL, N).transpose(1, 2, 0))
    return out
